# revision 2
# baseline (speedup 1.0000x reference)
"""Trainium2 Bass kernel for nn_Linear_67070209294813 (moe_routing).

Computes, for x:[B,S,Din] f32:
    base = x @ w_base.T + b_base
    gate = softmax(blend(x @ w_router_{img,text}.T + b_router), axis=E)
    h    = einsum("td,erd->ter", x, lora_A) * gate
    out  = base + einsum("ter,eor->to", h, lora_B) * SCALING

Strategy: data-parallel over the 8192 tokens across 8 NeuronCores (1024
tokens/core).  Per core one bf16 GEMM out^T[dout, tok] = sum_k
wT[k,dout-tile].T @ xT[k, tok] accumulated in fp32 PSUM, with the LoRA
rank-65 (64 rank dims + 1 bias row) matmul accumulated into the same PSUM
banks, so bias add and the base+lora sum cost nothing.  Inputs are cast
to bf16 on the host (error ~2^-9 * sqrt(K) << 2e-2 tolerance); weights
stream as 32 x 1MB contiguous panel DMAs (vs 1024 x 64KB in the fp32
version), so DMA (~95us) hides fully under PE time (~460us).
Routers/LoRA-A run as one small [din,72]-wide matmul; softmax runs in
token-partition layout via two tiny PE transposes; the gate is expanded
over the 16 ranks of each expert with a 0/1 replication matmul.

All operands are pre-transposed / blocked on the host so every DMA is
contiguous and the contraction dim lands on SBUF partitions.
"""

import sys

sys.path.insert(0, "/opt/trn_rl_repo")

import numpy as np
import ml_dtypes

import concourse.bass as bass  # noqa: F401  (bass must import before tile)
import concourse.mybir as mybir
import concourse.tile as tile
from concourse import bacc
from concourse.bass_utils import run_bass_kernel_spmd

B, S, D_IN, D_OUT = 4, 2048, 4096, 4096
R, E, SPLIT = 16, 4, 32
SCALING = 32.0 / 16.0
N_CORES = 8
TOK = B * S
TPC = TOK // N_CORES  # tokens per core
ER = E * R  # 64 rank dims across experts

F32 = mybir.dt.float32
BF16 = mybir.dt.bfloat16
NPBF16 = ml_dtypes.bfloat16
AF = mybir.ActivationFunctionType


def build_program(din, dout, tpc):
    """Emit + compile the per-core Tile program. Returns the Bacc object."""
    nk = din // 128  # k tiles (contraction)
    nm = dout // 128  # output-row tiles
    nt = tpc // 128  # token chunks (for the tiny softmax transposes)
    # moving-dim slices of the token axis (<=512 fp32 PSUM cols per matmul)
    n_sl = [(i, min(512, tpc - i)) for i in range(0, tpc, 512)]
    wr = 2 * E  # router logit columns (img then text)
    hcols = ER + wr  # 72: lora-A ranks + both routers
    XCH = 4  # x loaded in XCH chunks so phase-B can start early
    nkc = nk // XCH

    nc = bacc.Bacc("TRN2", target_bir_lowering=False, debug=False)

    xb = nc.dram_tensor("xb", [128, nk * tpc], BF16, kind="ExternalInput").ap()
    wkm = nc.dram_tensor("wkm", [nm, 128, nk * 128], BF16, kind="ExternalInput").ap()
    arb = nc.dram_tensor("arb", [128, nk * hcols], BF16, kind="ExternalInput").ap()
    bfb = nc.dram_tensor("bfb", [ER + 1, dout], BF16, kind="ExternalInput").ap()
    r4 = nc.dram_tensor("r4", [E, ER], BF16, kind="ExternalInput").ap()
    ones = nc.dram_tensor("ones", [1, tpc], BF16, kind="ExternalInput").ap()
    ident = nc.dram_tensor("ident", [128, 128], F32, kind="ExternalInput").ap()
    mask = nc.dram_tensor("mask", [128, nt], F32, kind="ExternalInput").ap()
    bbl = nc.dram_tensor("bbl", [128, nt * E], F32, kind="ExternalInput").ap()
    outT = nc.dram_tensor("outT", [dout, tpc], F32, kind="ExternalOutput").ap()

    with tile.TileContext(nc) as tc:
        with (
            tc.tile_pool(name="big", bufs=1) as big,
            tc.tile_pool(name="const", bufs=1) as const,
            tc.tile_pool(name="wp", bufs=3) as wp,
            tc.tile_pool(name="outp", bufs=2) as outp,
            tc.tile_pool(name="small", bufs=1) as small,
            tc.tile_pool(name="ps_main", bufs=2, space="PSUM") as ps_main,
            tc.tile_pool(name="ps_h", bufs=1, space="PSUM") as ps_h,
            tc.tile_pool(name="ps_t", bufs=2, space="PSUM") as ps_t,
        ):
            # ---- constants + x load -------------------------------------
            xt = big.tile([128, nk * tpc], BF16)
            for i in range(XCH):
                nc.sync.dma_start(
                    xt[:, i * nkc * tpc : (i + 1) * nkc * tpc],
                    xb[:, i * nkc * tpc : (i + 1) * nkc * tpc],
                )
            ar_sb = const.tile([128, nk * hcols], BF16)
            nc.sync.dma_start(ar_sb[:], arb[:, :])
            bf_sb = const.tile([ER + 1, dout], BF16)
            nc.sync.dma_start(bf_sb[:], bfb[:, :])
            r4_sb = const.tile([E, ER], BF16)
            nc.sync.dma_start(r4_sb[:], r4[:, :])
            id_sb = const.tile([128, 128], F32)
            nc.sync.dma_start(id_sb[:], ident[:, :])
            mask_sb = const.tile([128, nt], F32)
            nc.sync.dma_start(mask_sb[:], mask[:, :])
            bbl_sb = const.tile([128, nt * E], F32)
            nc.sync.dma_start(bbl_sb[:], bbl[:, :])

            # ---- phase B: hT[er,tok] + router logits, one wide matmul ---
            ph = ps_h.tile([hcols, tpc], F32, tag="h")
            for k in range(nk):
                lhs = ar_sb[:, k * hcols : (k + 1) * hcols]
                for o, w_ in n_sl:
                    nc.tensor.matmul(
                        ph[:, o : o + w_],
                        lhs,
                        xt[:, k * tpc + o : k * tpc + o + w_],
                        start=(k == 0),
                        stop=(k == nk - 1),
                    )
            hT = small.tile([hcols, tpc], F32)
            nc.vector.tensor_copy(hT[:], ph[:])
            lgT = small.tile([wr, tpc], F32)
            # partition-moving copy (rows ER..ER+wr -> 0..wr) must be a DMA
            nc.sync.dma_start(lgT[:], hT[ER : ER + wr, :])

            def base_kloop(m):
                ps = ps_main.tile([128, tpc], F32, tag="ps")
                wt = wp.tile([128, nk * 128], BF16, tag="w")
                nc.sync.dma_start(wt[:], wkm[m, :, :])
                for k in range(nk):
                    for o, w_ in n_sl:
                        nc.tensor.matmul(
                            ps[:, o : o + w_],
                            wt[:, k * 128 : (k + 1) * 128],
                            xt[:, k * tpc + o : k * tpc + o + w_],
                            start=(k == 0),
                            stop=False,
                        )
                return ps

            def lora_tail(m, ps, hw):
                for o, w_ in n_sl:
                    nc.tensor.matmul(
                        ps[:, o : o + w_],
                        bf_sb[:, m * 128 : (m + 1) * 128],
                        hw[:, o : o + w_],
                        start=False,
                        stop=True,
                    )
                ot = outp.tile([128, tpc], F32, tag="o")
                nc.vector.tensor_copy(ot[:], ps[:])
                nc.sync.dma_start(outT[m * 128 : (m + 1) * 128, :], ot[:])

            # m=0 base matmuls first so the PE stays busy while the (DVE)
            # softmax below runs; its lora tail is emitted after gating.
            ps0 = base_kloop(0)

            # ---- gating: softmax over E in token-partition layout -------
            lg = small.tile([128, nt * wr], F32)
            for t in range(nt):
                pt = ps_t.tile([128, wr], F32, tag="tp")
                nc.tensor.transpose(
                    pt[:], lgT[:, t * 128 : (t + 1) * 128], id_sb[0:wr, 0:wr]
                )
                nc.vector.tensor_copy(lg[:, t * wr : (t + 1) * wr], pt[:])
            lg3 = lg[:].rearrange("p (t j) -> p t j", j=wr)
            l_img, l_text = lg3[:, :, 0:E], lg3[:, :, E : 2 * E]
            g = small.tile([128, nt * E], F32)
            g3 = g[:].rearrange("p (t e) -> p t e", e=E)
            mb = mask_sb[:, :, None].broadcast_to([128, nt, E])
            nc.vector.tensor_sub(g3, l_img, l_text)
            nc.vector.tensor_mul(g3, g3, mb)
            nc.vector.tensor_add(g3, g3, l_text)
            nc.vector.tensor_add(g[:], g[:], bbl_sb[:])
            nc.scalar.activation(g[:], g[:], AF.Exp)
            zt = small.tile([128, nt], F32)
            nc.vector.reduce_sum(zt[:], g3, axis=mybir.AxisListType.X)
            nc.vector.reciprocal(zt[:], zt[:])
            nc.vector.tensor_mul(g3, g3, zt[:, :, None].broadcast_to([128, nt, E]))
            # gate back to [E, tok] layout, then expand across the 16 ranks
            g4 = small.tile([E, tpc], BF16)
            for t in range(nt):
                pt = ps_t.tile([E, 128], F32, tag="tp")
                nc.tensor.transpose(pt[:], g[:, t * E : (t + 1) * E], id_sb[:, :])
                nc.vector.tensor_copy(g4[:, t * 128 : (t + 1) * 128], pt[:])
            pgr = ps_h.tile([ER, tpc], F32, tag="h")
            for o, w_ in n_sl:
                nc.tensor.matmul(
                    pgr[:, o : o + w_], r4_sb[:], g4[:, o : o + w_],
                    start=True, stop=True,
                )
            hw = small.tile([ER + 1, tpc], BF16)
            nc.sync.dma_start(hw[ER : ER + 1, :], ones[:, :])
            nc.vector.tensor_mul(hw[0:ER, :], hT[0:ER, :], pgr[:])

            # ---- main GEMM over output-row tiles ------------------------
            lora_tail(0, ps0, hw)
            for m in range(1, nm):
                ps = base_kloop(m)
                lora_tail(m, ps, hw)

    nc.compile()
    return nc


def pack_inputs(
    x_flat, w_base, b_base, w_router_img, b_router_img, w_router_text,
    b_router_text, lora_A, lora_B, n_cores,
):
    """Host-side marshalling into the per-core DRAM layouts (bf16)."""
    tok, din = x_flat.shape
    dout = w_base.shape[0]
    tpc = tok // n_cores
    nk, nm, nt = din // 128, dout // 128, tpc // 128
    e, r = lora_A.shape[0], lora_A.shape[1]
    er = e * r
    hcols = er + 2 * e

    f32 = np.float32
    # wkm[m, p, k*128+j] = w_base[m*128+j, k*128+p]
    w4 = np.asarray(w_base, f32).astype(NPBF16).reshape(nm, 128, nk, 128)
    wkm = np.ascontiguousarray(w4.transpose(0, 3, 2, 1)).reshape(nm, 128, nk * 128)
    # arb[p, k*hcols + c] = [lora_A^T | w_router_img^T | w_router_text^T][k*128+p, c]
    arcat = np.concatenate(
        [lora_A.reshape(er, din).T, w_router_img.T, w_router_text.T], axis=1
    ).astype(NPBF16)  # [din, hcols]
    arb = np.ascontiguousarray(
        arcat.reshape(nk, 128, hcols).transpose(1, 0, 2)
    ).reshape(128, nk * hcols)
    bfm = (lora_B.transpose(0, 2, 1).reshape(er, dout) * SCALING).astype(f32)
    bfb = np.concatenate(
        [bfm, np.asarray(b_base, f32).reshape(1, dout)], axis=0
    ).astype(NPBF16)
    r4 = np.zeros((e, er), NPBF16)
    for i in range(e):
        r4[i, i * r : (i + 1) * r] = 1.0
    ident = np.eye(128, dtype=f32)

    shared = {
        "wkm": wkm, "arb": arb, "bfb": bfb, "r4": r4, "ident": ident,
        "ones": np.ones((1, tpc), NPBF16),
    }
    in_maps = []
    for c in range(n_cores):
        xc = x_flat[c * tpc : (c + 1) * tpc]  # [tpc, din] f32
        # xb[p, k*tpc + t] = xc[t, k*128+p]
        xbc = np.ascontiguousarray(
            xc.astype(NPBF16).reshape(tpc, nk, 128).transpose(2, 1, 0)
        ).reshape(128, nk * tpc)
        toks = c * tpc + np.arange(tpc)
        m = ((toks % S) < SPLIT).astype(f32)  # image-token mask
        mask_pc = np.ascontiguousarray(m.reshape(nt, 128).T)  # [128, nt]
        bb = (
            m[:, None] * np.asarray(b_router_img, f32)[None, :]
            + (1.0 - m[:, None]) * np.asarray(b_router_text, f32)[None, :]
        )  # [tpc, e]
        bbl_pc = np.ascontiguousarray(
            bb.reshape(nt, 128, e).transpose(1, 0, 2)
        ).reshape(128, nt * e)
        in_maps.append({"xb": xbc, "mask": mask_pc, "bbl": bbl_pc, **shared})
    return in_maps


_prog_cache = {}


def _get_program():
    key = (D_IN, D_OUT, TPC)
    if key not in _prog_cache:
        _prog_cache[key] = build_program(D_IN, D_OUT, TPC)
    return _prog_cache[key]


def kernel(
    x, w_base, b_base, w_router_img, b_router_img, w_router_text,
    b_router_text, lora_A, lora_B,
):
    x = np.asarray(x, dtype=np.float32)
    x_flat = np.ascontiguousarray(x.reshape(TOK, D_IN))
    in_maps = pack_inputs(
        x_flat, np.asarray(w_base, np.float32), np.asarray(b_base, np.float32),
        np.asarray(w_router_img, np.float32), np.asarray(b_router_img, np.float32),
        np.asarray(w_router_text, np.float32), np.asarray(b_router_text, np.float32),
        np.asarray(lora_A, np.float32), np.asarray(lora_B, np.float32),
        N_CORES,
    )
    nc = _get_program()
    out = np.empty((TOK, D_OUT), np.float32)
    # First execution after a fresh device open has (rarely) returned
    # corrupt data; retry once if the result is wildly out of range.
    for attempt in range(3):
        res = run_bass_kernel_spmd(nc, in_maps, core_ids=list(range(N_CORES)))
        for c in range(N_CORES):
            out[c * TPC : (c + 1) * TPC, :] = res.results[c]["outT"].T
        if np.isfinite(out).all() and np.abs(out).max() < 1e3:
            break
    return out.reshape(B, S, D_OUT)


# revision 4
# speedup vs baseline: 1.3435x; 1.3435x over previous
"""Trainium2 Bass kernel for nn_Linear_67070209294813 (moe_routing).

Computes, for x:[B,S,Din] f32:
    base = x @ w_base.T + b_base
    gate = softmax(blend(x @ w_router_{img,text}.T + b_router), axis=E)
    h    = einsum("td,erd->ter", x, lora_A) * gate
    out  = base + einsum("ter,eor->to", h, lora_B) * SCALING

Strategy: data-parallel over the 8192 tokens across 8 NeuronCores (1024
tokens/core).  Per core one bf16 GEMM out^T[dout, tok] = sum_k
wT[k,dout-tile].T @ xT[k, tok] accumulated in fp32 PSUM, with the LoRA
rank-65 (64 rank dims + 1 bias row) matmul accumulated into the same PSUM
banks, so bias add and the base+lora sum cost nothing.  Inputs are cast
to bf16 on the host (error ~2^-9 * sqrt(K) << 2e-2 tolerance); weights
stream as 32 x 1MB contiguous panel DMAs (vs 1024 x 64KB in the fp32
version), so DMA (~95us) hides fully under PE time (~460us).
Routers/LoRA-A run as one small [din,72]-wide matmul; softmax runs in
token-partition layout via two tiny PE transposes; the gate is expanded
over the 16 ranks of each expert with a 0/1 replication matmul.

All operands are pre-transposed / blocked on the host so every DMA is
contiguous and the contraction dim lands on SBUF partitions.
"""

import sys

sys.path.insert(0, "/opt/trn_rl_repo")

import numpy as np
import ml_dtypes

import concourse.bass as bass  # noqa: F401  (bass must import before tile)
import concourse.mybir as mybir
import concourse.tile as tile
from concourse import bacc
from concourse.bass_utils import run_bass_kernel_spmd

B, S, D_IN, D_OUT = 4, 2048, 4096, 4096
R, E, SPLIT = 16, 4, 32
SCALING = 32.0 / 16.0
N_CORES = 8
TOK = B * S
TPC = TOK // N_CORES  # tokens per core
ER = E * R  # 64 rank dims across experts

F32 = mybir.dt.float32
BF16 = mybir.dt.bfloat16
NPBF16 = ml_dtypes.bfloat16
AF = mybir.ActivationFunctionType


def build_program(din, dout, tpc):
    """Emit + compile the per-core Tile program. Returns the Bacc object."""
    nk = din // 128  # k tiles (contraction)
    nm = dout // 128  # output-row tiles
    nt = tpc // 128  # token chunks (for the tiny softmax transposes)
    # moving-dim slices of the token axis (<=512 fp32 PSUM cols per matmul)
    n_sl = [(i, min(512, tpc - i)) for i in range(0, tpc, 512)]
    wr = 2 * E  # router logit columns (img then text)
    hcols = ER + wr  # 72: lora-A ranks + both routers
    XCH = 4  # x loaded in XCH chunks so phase-B can start early
    nkc = nk // XCH

    nc = bacc.Bacc("TRN2", target_bir_lowering=False, debug=False)

    # consolidated operands: fewer per-call buffer bindings (the axon
    # dispatch path charges ~8us per operand per call)
    cb_ar, cb_bf, cb_r4, cb_on = 0, nk * hcols, nk * hcols + dout, nk * hcols + dout + ER
    CB = cb_on + tpc
    cf_id, cf_mk, cf_bb = 0, 128, 128 + nt
    CF = 128 + nt + nt * E
    xb = nc.dram_tensor("xb", [128, nk * tpc], BF16, kind="ExternalInput").ap()
    wkm = nc.dram_tensor("wkm", [nm, 128, nk * 128], BF16, kind="ExternalInput").ap()
    cstb = nc.dram_tensor("cstb", [128, CB], BF16, kind="ExternalInput").ap()
    cstf = nc.dram_tensor("cstf", [128, CF], F32, kind="ExternalInput").ap()
    outT = nc.dram_tensor("outT", [dout, tpc], F32, kind="ExternalOutput").ap()

    with tile.TileContext(nc) as tc:
        with (
            tc.tile_pool(name="big", bufs=1) as big,
            tc.tile_pool(name="const", bufs=1) as const,
            tc.tile_pool(name="wp", bufs=3) as wp,
            tc.tile_pool(name="outp", bufs=2) as outp,
            tc.tile_pool(name="small", bufs=1) as small,
            tc.tile_pool(name="ps_main", bufs=2, space="PSUM") as ps_main,
            tc.tile_pool(name="ps_h", bufs=1, space="PSUM") as ps_h,
            tc.tile_pool(name="ps_t", bufs=2, space="PSUM") as ps_t,
        ):
            # ---- constants + x load -------------------------------------
            xt = big.tile([128, nk * tpc], BF16)
            for i in range(XCH):
                nc.sync.dma_start(
                    xt[:, i * nkc * tpc : (i + 1) * nkc * tpc],
                    xb[:, i * nkc * tpc : (i + 1) * nkc * tpc],
                )
            ar_sb = const.tile([128, nk * hcols], BF16)
            nc.sync.dma_start(ar_sb[:], cstb[:, cb_ar : cb_ar + nk * hcols])
            bf_sb = const.tile([ER + 1, dout], BF16)
            nc.sync.dma_start(bf_sb[:], cstb[0 : ER + 1, cb_bf : cb_bf + dout])
            r4_sb = const.tile([E, ER], BF16)
            nc.sync.dma_start(r4_sb[:], cstb[0:E, cb_r4 : cb_r4 + ER])
            id_sb = const.tile([128, 128], F32)
            nc.sync.dma_start(id_sb[:], cstf[:, cf_id : cf_id + 128])
            mask_sb = const.tile([128, nt], F32)
            nc.sync.dma_start(mask_sb[:], cstf[:, cf_mk : cf_mk + nt])
            bbl_sb = const.tile([128, nt * E], F32)
            nc.sync.dma_start(bbl_sb[:], cstf[:, cf_bb : cf_bb + nt * E])

            # ---- phase B: hT[er,tok] + router logits, one wide matmul ---
            ph = ps_h.tile([hcols, tpc], F32, tag="h")
            for k in range(nk):
                lhs = ar_sb[:, k * hcols : (k + 1) * hcols]
                for o, w_ in n_sl:
                    nc.tensor.matmul(
                        ph[:, o : o + w_],
                        lhs,
                        xt[:, k * tpc + o : k * tpc + o + w_],
                        start=(k == 0),
                        stop=(k == nk - 1),
                    )
            hT = small.tile([hcols, tpc], F32)
            nc.vector.tensor_copy(hT[:], ph[:])
            lgT = small.tile([wr, tpc], F32)
            # partition-moving copy (rows ER..ER+wr -> 0..wr) must be a DMA
            nc.sync.dma_start(lgT[:], hT[ER : ER + wr, :])

            def base_kloop(m):
                ps = ps_main.tile([128, tpc], F32, tag="ps")
                wt = wp.tile([128, nk * 128], BF16, tag="w")
                nc.sync.dma_start(wt[:], wkm[m, :, :])
                for k in range(nk):
                    for o, w_ in n_sl:
                        nc.tensor.matmul(
                            ps[:, o : o + w_],
                            wt[:, k * 128 : (k + 1) * 128],
                            xt[:, k * tpc + o : k * tpc + o + w_],
                            start=(k == 0),
                            stop=False,
                        )
                return ps

            def lora_tail(m, ps, hw):
                for o, w_ in n_sl:
                    nc.tensor.matmul(
                        ps[:, o : o + w_],
                        bf_sb[:, m * 128 : (m + 1) * 128],
                        hw[:, o : o + w_],
                        start=False,
                        stop=True,
                    )
                ot = outp.tile([128, tpc], F32, tag="o")
                nc.vector.tensor_copy(ot[:], ps[:])
                nc.sync.dma_start(outT[m * 128 : (m + 1) * 128, :], ot[:])

            # m=0 base matmuls first so the PE stays busy while the (DVE)
            # softmax below runs; its lora tail is emitted after gating.
            ps0 = base_kloop(0)

            # ---- gating: softmax over E in token-partition layout -------
            lg = small.tile([128, nt * wr], F32)
            for t in range(nt):
                pt = ps_t.tile([128, wr], F32, tag="tp")
                nc.tensor.transpose(
                    pt[:], lgT[:, t * 128 : (t + 1) * 128], id_sb[0:wr, 0:wr]
                )
                nc.vector.tensor_copy(lg[:, t * wr : (t + 1) * wr], pt[:])
            lg3 = lg[:].rearrange("p (t j) -> p t j", j=wr)
            l_img, l_text = lg3[:, :, 0:E], lg3[:, :, E : 2 * E]
            g = small.tile([128, nt * E], F32)
            g3 = g[:].rearrange("p (t e) -> p t e", e=E)
            mb = mask_sb[:, :, None].broadcast_to([128, nt, E])
            nc.vector.tensor_sub(g3, l_img, l_text)
            nc.vector.tensor_mul(g3, g3, mb)
            nc.vector.tensor_add(g3, g3, l_text)
            nc.vector.tensor_add(g[:], g[:], bbl_sb[:])
            nc.scalar.activation(g[:], g[:], AF.Exp)
            zt = small.tile([128, nt], F32)
            nc.vector.reduce_sum(zt[:], g3, axis=mybir.AxisListType.X)
            nc.vector.reciprocal(zt[:], zt[:])
            nc.vector.tensor_mul(g3, g3, zt[:, :, None].broadcast_to([128, nt, E]))
            # gate back to [E, tok] layout, then expand across the 16 ranks
            g4 = small.tile([E, tpc], BF16)
            for t in range(nt):
                pt = ps_t.tile([E, 128], F32, tag="tp")
                nc.tensor.transpose(pt[:], g[:, t * E : (t + 1) * E], id_sb[:, :])
                nc.vector.tensor_copy(g4[:, t * 128 : (t + 1) * 128], pt[:])
            pgr = ps_h.tile([ER, tpc], F32, tag="h")
            for o, w_ in n_sl:
                nc.tensor.matmul(
                    pgr[:, o : o + w_], r4_sb[:], g4[:, o : o + w_],
                    start=True, stop=True,
                )
            hw = small.tile([ER + 1, tpc], BF16)
            nc.sync.dma_start(hw[ER : ER + 1, :], cstb[0:1, cb_on : cb_on + tpc])
            nc.vector.tensor_mul(hw[0:ER, :], hT[0:ER, :], pgr[:])

            # ---- main GEMM over output-row tiles ------------------------
            lora_tail(0, ps0, hw)
            for m in range(1, nm):
                ps = base_kloop(m)
                lora_tail(m, ps, hw)

    nc.compile()
    return nc


def pack_inputs(
    x_flat, w_base, b_base, w_router_img, b_router_img, w_router_text,
    b_router_text, lora_A, lora_B, n_cores,
):
    """Host-side marshalling into the per-core DRAM layouts (bf16)."""
    tok, din = x_flat.shape
    dout = w_base.shape[0]
    tpc = tok // n_cores
    nk, nm, nt = din // 128, dout // 128, tpc // 128
    e, r = lora_A.shape[0], lora_A.shape[1]
    er = e * r
    hcols = er + 2 * e

    f32 = np.float32
    # wkm[m, p, k*128+j] = w_base[m*128+j, k*128+p]
    w4 = np.asarray(w_base, f32).astype(NPBF16).reshape(nm, 128, nk, 128)
    wkm = np.ascontiguousarray(w4.transpose(0, 3, 2, 1)).reshape(nm, 128, nk * 128)
    # arb[p, k*hcols + c] = [lora_A^T | w_router_img^T | w_router_text^T][k*128+p, c]
    arcat = np.concatenate(
        [lora_A.reshape(er, din).T, w_router_img.T, w_router_text.T], axis=1
    ).astype(NPBF16)  # [din, hcols]
    arb = np.ascontiguousarray(
        arcat.reshape(nk, 128, hcols).transpose(1, 0, 2)
    ).reshape(128, nk * hcols)
    bfm = (lora_B.transpose(0, 2, 1).reshape(er, dout) * SCALING).astype(f32)
    bfb = np.concatenate(
        [bfm, np.asarray(b_base, f32).reshape(1, dout)], axis=0
    ).astype(NPBF16)
    # consolidated bf16 constant block (layout must match build_program)
    cb_ar, cb_bf, cb_r4, cb_on = 0, nk * hcols, nk * hcols + dout, nk * hcols + dout + er
    CB = cb_on + tpc
    cstb = np.zeros((128, CB), NPBF16)
    cstb[:, cb_ar : cb_ar + nk * hcols] = arb
    cstb[0 : er + 1, cb_bf : cb_bf + dout] = bfb
    for i in range(e):
        cstb[i, cb_r4 + i * r : cb_r4 + (i + 1) * r] = 1.0
    cstb[0, cb_on : cb_on + tpc] = 1.0

    in_maps = []
    for c in range(n_cores):
        xc = x_flat[c * tpc : (c + 1) * tpc]  # [tpc, din] f32
        # xb[p, k*tpc + t] = xc[t, k*128+p]
        xbc = np.ascontiguousarray(
            xc.astype(NPBF16).reshape(tpc, nk, 128).transpose(2, 1, 0)
        ).reshape(128, nk * tpc)
        toks = c * tpc + np.arange(tpc)
        m = ((toks % S) < SPLIT).astype(f32)  # image-token mask
        mask_pc = np.ascontiguousarray(m.reshape(nt, 128).T)  # [128, nt]
        bb = (
            m[:, None] * np.asarray(b_router_img, f32)[None, :]
            + (1.0 - m[:, None]) * np.asarray(b_router_text, f32)[None, :]
        )  # [tpc, e]
        bbl_pc = np.ascontiguousarray(
            bb.reshape(nt, 128, e).transpose(1, 0, 2)
        ).reshape(128, nt * e)
        # consolidated f32 constant block: ident | mask | bbl
        CF = 128 + nt + nt * e
        cstf = np.zeros((128, CF), f32)
        cstf[:, 0:128] = np.eye(128, dtype=f32)
        cstf[:, 128 : 128 + nt] = mask_pc
        cstf[:, 128 + nt :] = bbl_pc
        in_maps.append({"xb": xbc, "cstb": cstb, "cstf": cstf, "wkm": wkm})
    return in_maps


_prog_cache = {}


def _get_program():
    key = (D_IN, D_OUT, TPC)
    if key not in _prog_cache:
        _prog_cache[key] = build_program(D_IN, D_OUT, TPC)
    return _prog_cache[key]


def kernel(
    x, w_base, b_base, w_router_img, b_router_img, w_router_text,
    b_router_text, lora_A, lora_B,
):
    x = np.asarray(x, dtype=np.float32)
    x_flat = np.ascontiguousarray(x.reshape(TOK, D_IN))
    in_maps = pack_inputs(
        x_flat, np.asarray(w_base, np.float32), np.asarray(b_base, np.float32),
        np.asarray(w_router_img, np.float32), np.asarray(b_router_img, np.float32),
        np.asarray(w_router_text, np.float32), np.asarray(b_router_text, np.float32),
        np.asarray(lora_A, np.float32), np.asarray(lora_B, np.float32),
        N_CORES,
    )
    nc = _get_program()
    out = np.empty((TOK, D_OUT), np.float32)
    # First execution after a fresh device open has (rarely) returned
    # corrupt data; retry once if the result is wildly out of range.
    for attempt in range(3):
        res = run_bass_kernel_spmd(nc, in_maps, core_ids=list(range(N_CORES)))
        for c in range(N_CORES):
            out[c * TPC : (c + 1) * TPC, :] = res.results[c]["outT"].T
        if np.isfinite(out).all() and np.abs(out).max() < 1e3:
            break
    return out.reshape(B, S, D_OUT)


# revision 5
# speedup vs baseline: 1.4923x; 1.1108x over previous
"""Trainium2 Bass kernel for nn_Linear_67070209294813 (moe_routing).

Computes, for x:[B,S,Din] f32:
    base = x @ w_base.T + b_base
    gate = softmax(blend(x @ w_router_{img,text}.T + b_router), axis=E)
    h    = einsum("td,erd->ter", x, lora_A) * gate
    out  = base + einsum("ter,eor->to", h, lora_B) * SCALING

Strategy: data-parallel over the 8192 tokens across 8 NeuronCores (1024
tokens/core).  Per core one bf16 GEMM out^T[dout, tok] = sum_k
wT[k,dout-tile].T @ xT[k, tok] accumulated in fp32 PSUM, with the LoRA
rank-65 (64 rank dims + 1 bias row) matmul accumulated into the same PSUM
banks, so bias add and the base+lora sum cost nothing.  Inputs are cast
to bf16 on the host (error ~2^-9 * sqrt(K) << 2e-2 tolerance); weights
stream as 32 x 1MB contiguous panel DMAs (vs 1024 x 64KB in the fp32
version), so DMA (~95us) hides fully under PE time (~460us).
Routers/LoRA-A run as one small [din,72]-wide matmul; softmax runs in
token-partition layout via two tiny PE transposes; the gate is expanded
over the 16 ranks of each expert with a 0/1 replication matmul.

All operands are pre-transposed / blocked on the host so every DMA is
contiguous and the contraction dim lands on SBUF partitions.
"""

import sys

sys.path.insert(0, "/opt/trn_rl_repo")

import numpy as np
import ml_dtypes

import concourse.bass as bass  # noqa: F401  (bass must import before tile)
import concourse.mybir as mybir
import concourse.tile as tile
from concourse import bacc
from concourse.bass_utils import run_bass_kernel_spmd

B, S, D_IN, D_OUT = 4, 2048, 4096, 4096
R, E, SPLIT = 16, 4, 32
SCALING = 32.0 / 16.0
N_CORES = 8
TOK = B * S
TPC = TOK // N_CORES  # tokens per core
ER = E * R  # 64 rank dims across experts

F32 = mybir.dt.float32
BF16 = mybir.dt.bfloat16
NPBF16 = ml_dtypes.bfloat16
AF = mybir.ActivationFunctionType


def build_program(din, dout, tpc):
    """Emit + compile the per-core Tile program. Returns the Bacc object."""
    nk = din // 128  # k tiles (contraction)
    nm = dout // 128  # output-row tiles
    nt = tpc // 128  # token chunks (for the tiny softmax transposes)
    # moving-dim slices of the token axis (<=512 fp32 PSUM cols per matmul)
    n_sl = [(i, min(512, tpc - i)) for i in range(0, tpc, 512)]
    wr = 2 * E  # router logit columns (img then text)
    hcols = ER + wr  # 72: lora-A ranks + both routers
    XCH = 4  # x loaded in XCH chunks so phase-B can start early
    nkc = nk // XCH

    nc = bacc.Bacc("TRN2", target_bir_lowering=False, debug=False)

    # consolidated operands: fewer per-call buffer bindings (the axon
    # dispatch path charges ~8us per operand per call)
    cb_ar, cb_bf, cb_r4, cb_on = 0, nk * hcols, nk * hcols + dout, nk * hcols + dout + ER
    cb_cf = cb_on + tpc  # f32 block (ident|mask|bbl) stored as 2x bf16 cols
    cf_id, cf_mk, cf_bb = 0, 128, 128 + nt
    CF = 128 + nt + nt * E
    CB = cb_cf + 2 * CF
    xb = nc.dram_tensor("xb", [128, nk * tpc], BF16, kind="ExternalInput").ap()
    wkm = nc.dram_tensor("wkm", [nm, 128, nk * 128], BF16, kind="ExternalInput").ap()
    cstb = nc.dram_tensor("cstb", [128, CB], BF16, kind="ExternalInput").ap()
    outT = nc.dram_tensor("outT", [dout, tpc], F32, kind="ExternalOutput").ap()

    with tile.TileContext(nc) as tc:
        with (
            tc.tile_pool(name="big", bufs=1) as big,
            tc.tile_pool(name="const", bufs=1) as const,
            tc.tile_pool(name="wp", bufs=3) as wp,
            tc.tile_pool(name="outp", bufs=2) as outp,
            tc.tile_pool(name="small", bufs=1) as small,
            tc.tile_pool(name="ps_main", bufs=2, space="PSUM") as ps_main,
            tc.tile_pool(name="ps_h", bufs=1, space="PSUM") as ps_h,
            tc.tile_pool(name="ps_t", bufs=2, space="PSUM") as ps_t,
        ):
            # ---- constants + x load -------------------------------------
            xt = big.tile([128, nk * tpc], BF16)
            for i in range(XCH):
                nc.sync.dma_start(
                    xt[:, i * nkc * tpc : (i + 1) * nkc * tpc],
                    xb[:, i * nkc * tpc : (i + 1) * nkc * tpc],
                )
            ar_sb = const.tile([128, nk * hcols], BF16)
            nc.sync.dma_start(ar_sb[:], cstb[:, cb_ar : cb_ar + nk * hcols])
            bf_sb = const.tile([ER + 1, dout], BF16)
            nc.sync.dma_start(bf_sb[:], cstb[0 : ER + 1, cb_bf : cb_bf + dout])
            r4_sb = const.tile([E, ER], BF16)
            nc.sync.dma_start(r4_sb[:], cstb[0:E, cb_r4 : cb_r4 + ER])
            id_sb = const.tile([128, 128], F32)
            nc.sync.dma_start(
                id_sb[:],
                cstb[:, cb_cf + 2 * cf_id : cb_cf + 2 * (cf_id + 128)].bitcast(F32),
            )
            mask_sb = const.tile([128, nt], F32)
            nc.sync.dma_start(
                mask_sb[:],
                cstb[:, cb_cf + 2 * cf_mk : cb_cf + 2 * (cf_mk + nt)].bitcast(F32),
            )
            bbl_sb = const.tile([128, nt * E], F32)
            nc.sync.dma_start(
                bbl_sb[:],
                cstb[:, cb_cf + 2 * cf_bb : cb_cf + 2 * (cf_bb + nt * E)].bitcast(F32),
            )

            # ---- phase B: hT[er,tok] + router logits, one wide matmul ---
            ph = ps_h.tile([hcols, tpc], F32, tag="h")
            for k in range(nk):
                lhs = ar_sb[:, k * hcols : (k + 1) * hcols]
                for o, w_ in n_sl:
                    nc.tensor.matmul(
                        ph[:, o : o + w_],
                        lhs,
                        xt[:, k * tpc + o : k * tpc + o + w_],
                        start=(k == 0),
                        stop=(k == nk - 1),
                    )
            hT = small.tile([hcols, tpc], F32)
            nc.vector.tensor_copy(hT[:], ph[:])
            lgT = small.tile([wr, tpc], F32)
            # partition-moving copy (rows ER..ER+wr -> 0..wr) must be a DMA
            nc.sync.dma_start(lgT[:], hT[ER : ER + wr, :])

            def base_kloop(m):
                ps = ps_main.tile([128, tpc], F32, tag="ps")
                wt = wp.tile([128, nk * 128], BF16, tag="w")
                nc.sync.dma_start(wt[:], wkm[m, :, :])
                for k in range(nk):
                    for o, w_ in n_sl:
                        nc.tensor.matmul(
                            ps[:, o : o + w_],
                            wt[:, k * 128 : (k + 1) * 128],
                            xt[:, k * tpc + o : k * tpc + o + w_],
                            start=(k == 0),
                            stop=False,
                        )
                return ps

            def lora_tail(m, ps, hw):
                for o, w_ in n_sl:
                    nc.tensor.matmul(
                        ps[:, o : o + w_],
                        bf_sb[:, m * 128 : (m + 1) * 128],
                        hw[:, o : o + w_],
                        start=False,
                        stop=True,
                    )
                ot = outp.tile([128, tpc], F32, tag="o")
                nc.vector.tensor_copy(ot[:], ps[:])
                nc.sync.dma_start(outT[m * 128 : (m + 1) * 128, :], ot[:])

            # m=0 base matmuls first so the PE stays busy while the (DVE)
            # softmax below runs; its lora tail is emitted after gating.
            ps0 = base_kloop(0)

            # ---- gating: softmax over E in token-partition layout -------
            lg = small.tile([128, nt * wr], F32)
            for t in range(nt):
                pt = ps_t.tile([128, wr], F32, tag="tp")
                nc.tensor.transpose(
                    pt[:], lgT[:, t * 128 : (t + 1) * 128], id_sb[0:wr, 0:wr]
                )
                nc.vector.tensor_copy(lg[:, t * wr : (t + 1) * wr], pt[:])
            lg3 = lg[:].rearrange("p (t j) -> p t j", j=wr)
            l_img, l_text = lg3[:, :, 0:E], lg3[:, :, E : 2 * E]
            g = small.tile([128, nt * E], F32)
            g3 = g[:].rearrange("p (t e) -> p t e", e=E)
            mb = mask_sb[:, :, None].broadcast_to([128, nt, E])
            nc.vector.tensor_sub(g3, l_img, l_text)
            nc.vector.tensor_mul(g3, g3, mb)
            nc.vector.tensor_add(g3, g3, l_text)
            nc.vector.tensor_add(g[:], g[:], bbl_sb[:])
            nc.scalar.activation(g[:], g[:], AF.Exp)
            zt = small.tile([128, nt], F32)
            nc.vector.reduce_sum(zt[:], g3, axis=mybir.AxisListType.X)
            nc.vector.reciprocal(zt[:], zt[:])
            nc.vector.tensor_mul(g3, g3, zt[:, :, None].broadcast_to([128, nt, E]))
            # gate back to [E, tok] layout, then expand across the 16 ranks
            g4 = small.tile([E, tpc], BF16)
            for t in range(nt):
                pt = ps_t.tile([E, 128], F32, tag="tp")
                nc.tensor.transpose(pt[:], g[:, t * E : (t + 1) * E], id_sb[:, :])
                nc.vector.tensor_copy(g4[:, t * 128 : (t + 1) * 128], pt[:])
            pgr = ps_h.tile([ER, tpc], F32, tag="h")
            for o, w_ in n_sl:
                nc.tensor.matmul(
                    pgr[:, o : o + w_], r4_sb[:], g4[:, o : o + w_],
                    start=True, stop=True,
                )
            hw = small.tile([ER + 1, tpc], BF16)
            nc.sync.dma_start(hw[ER : ER + 1, :], cstb[0:1, cb_on : cb_on + tpc])
            nc.vector.tensor_mul(hw[0:ER, :], hT[0:ER, :], pgr[:])

            # ---- main GEMM over output-row tiles ------------------------
            lora_tail(0, ps0, hw)
            for m in range(1, nm):
                ps = base_kloop(m)
                lora_tail(m, ps, hw)

    nc.compile()
    return nc


def pack_inputs(
    x_flat, w_base, b_base, w_router_img, b_router_img, w_router_text,
    b_router_text, lora_A, lora_B, n_cores,
):
    """Host-side marshalling into the per-core DRAM layouts (bf16)."""
    tok, din = x_flat.shape
    dout = w_base.shape[0]
    tpc = tok // n_cores
    nk, nm, nt = din // 128, dout // 128, tpc // 128
    e, r = lora_A.shape[0], lora_A.shape[1]
    er = e * r
    hcols = er + 2 * e

    f32 = np.float32
    # wkm[m, p, k*128+j] = w_base[m*128+j, k*128+p]
    w4 = np.asarray(w_base, f32).astype(NPBF16).reshape(nm, 128, nk, 128)
    wkm = np.ascontiguousarray(w4.transpose(0, 3, 2, 1)).reshape(nm, 128, nk * 128)
    # arb[p, k*hcols + c] = [lora_A^T | w_router_img^T | w_router_text^T][k*128+p, c]
    arcat = np.concatenate(
        [lora_A.reshape(er, din).T, w_router_img.T, w_router_text.T], axis=1
    ).astype(NPBF16)  # [din, hcols]
    arb = np.ascontiguousarray(
        arcat.reshape(nk, 128, hcols).transpose(1, 0, 2)
    ).reshape(128, nk * hcols)
    bfm = (lora_B.transpose(0, 2, 1).reshape(er, dout) * SCALING).astype(f32)
    bfb = np.concatenate(
        [bfm, np.asarray(b_base, f32).reshape(1, dout)], axis=0
    ).astype(NPBF16)
    # consolidated bf16 constant block (layout must match build_program)
    cb_ar, cb_bf, cb_r4, cb_on = 0, nk * hcols, nk * hcols + dout, nk * hcols + dout + er
    cb_cf = cb_on + tpc
    CF = 128 + nt + nt * e
    CB = cb_cf + 2 * CF
    cstb = np.zeros((128, CB), NPBF16)
    cstb[:, cb_ar : cb_ar + nk * hcols] = arb
    cstb[0 : er + 1, cb_bf : cb_bf + dout] = bfb
    for i in range(e):
        cstb[i, cb_r4 + i * r : cb_r4 + (i + 1) * r] = 1.0
    cstb[0, cb_on : cb_on + tpc] = 1.0

    in_maps = []
    for c in range(n_cores):
        xc = x_flat[c * tpc : (c + 1) * tpc]  # [tpc, din] f32
        # xb[p, k*tpc + t] = xc[t, k*128+p]
        xbc = np.ascontiguousarray(
            xc.astype(NPBF16).reshape(tpc, nk, 128).transpose(2, 1, 0)
        ).reshape(128, nk * tpc)
        toks = c * tpc + np.arange(tpc)
        m = ((toks % S) < SPLIT).astype(f32)  # image-token mask
        mask_pc = np.ascontiguousarray(m.reshape(nt, 128).T)  # [128, nt]
        bb = (
            m[:, None] * np.asarray(b_router_img, f32)[None, :]
            + (1.0 - m[:, None]) * np.asarray(b_router_text, f32)[None, :]
        )  # [tpc, e]
        bbl_pc = np.ascontiguousarray(
            bb.reshape(nt, 128, e).transpose(1, 0, 2)
        ).reshape(128, nt * e)
        # consolidated f32 constant block: ident | mask | bbl, bitcast into
        # the shared bf16 tensor's tail (per-core copy since mask/bbl differ)
        cstf = np.zeros((128, CF), f32)
        cstf[:, 0:128] = np.eye(128, dtype=f32)
        cstf[:, 128 : 128 + nt] = mask_pc
        cstf[:, 128 + nt :] = bbl_pc
        cstb_c = cstb.copy()
        cstb_c[:, cb_cf:] = cstf.view(NPBF16)
        in_maps.append({"xb": xbc, "cstb": cstb_c, "wkm": wkm})
    return in_maps


_prog_cache = {}


def _get_program():
    key = (D_IN, D_OUT, TPC)
    if key not in _prog_cache:
        _prog_cache[key] = build_program(D_IN, D_OUT, TPC)
    return _prog_cache[key]


def kernel(
    x, w_base, b_base, w_router_img, b_router_img, w_router_text,
    b_router_text, lora_A, lora_B,
):
    x = np.asarray(x, dtype=np.float32)
    x_flat = np.ascontiguousarray(x.reshape(TOK, D_IN))
    in_maps = pack_inputs(
        x_flat, np.asarray(w_base, np.float32), np.asarray(b_base, np.float32),
        np.asarray(w_router_img, np.float32), np.asarray(b_router_img, np.float32),
        np.asarray(w_router_text, np.float32), np.asarray(b_router_text, np.float32),
        np.asarray(lora_A, np.float32), np.asarray(lora_B, np.float32),
        N_CORES,
    )
    nc = _get_program()
    out = np.empty((TOK, D_OUT), np.float32)
    # First execution after a fresh device open has (rarely) returned
    # corrupt data; retry once if the result is wildly out of range.
    for attempt in range(3):
        res = run_bass_kernel_spmd(nc, in_maps, core_ids=list(range(N_CORES)))
        for c in range(N_CORES):
            out[c * TPC : (c + 1) * TPC, :] = res.results[c]["outT"].T
        if np.isfinite(out).all() and np.abs(out).max() < 1e3:
            break
    return out.reshape(B, S, D_OUT)
